# revision 1
# baseline (speedup 1.0000x reference)
"""Trainium2 Bass kernel for 4-head spatial self-attention.

Computation (per batch b):
    xf = x[b] reshaped [C=256, n=4096]
    q/k/v = Wq/Wk/Wv @ xf            -> [128, n]   (rows = 4 heads x 32 dims)
    S_h   = (q_h^T k_h) * 32^-0.5    -> [n, n] per head
    P     = exp(S)  (softmax without max-subtraction: logits are O(5), safe in fp32)
    A_h   = v_h @ P_h^T / rowsum     -> [32, n]
    out   = Wout @ A + bout          -> [C, n]

Sharding: 8 cores = 4 batches x 2 query-halves. Each core handles all 4 heads
for one batch and 2048 queries vs all 4096 keys; outputs are disjoint slices.

Device layout (no transposes anywhere):
 - S is computed TRANSPOSED (keys on partitions, queries free) with the d=32
   head contractions packed onto PE row strips via tile_position (32h, 0).
   HW constraint (probed): concurrent row-strip matmuls MUST write different
   PSUM banks, so heads run in pairs and each head's S^T tile gets its own
   bank ([128, 2, 512] st tile).
 - exp(SCALE * S^T) runs on ScalarE PSUM->SBUF (the bottleneck: ~33.5M
   elements/core at 1 elem/lane/cycle, FD=1024 per op).
 - P@V accumulates over key chunks (K=128) on PE col strips; the PV weights
   (pre-transposed v from the projection x^T @ Wv^T) carry an extra ones
   column (M=33), so row 32 of each strip accumulates the softmax denominator
   for free. A K=1 matmul then broadcasts that row across partitions and a
   DVE divide normalizes.
"""

import numpy as np
import sys

for _p in ("/opt/trn_rl_repo", "/opt/pypackages"):
    if _p not in sys.path:
        sys.path.append(_p)

import concourse.bass as bass
import concourse.tile as tile
from concourse import bacc, mybir
from concourse.tile import add_dep_helper
from concourse.bass_utils import run_bass_kernel_spmd

f32 = mybir.dt.float32

B = 4
C = 256
N = 4096          # h*w = 64*64 key positions
NQ = 2048         # queries per core (half batch)
HEADS = 4
DH = 32
INNER = 128
SCALE = DH ** -0.5

QB = 512          # query block (free dim of S^T tiles)
NQB = NQ // QB    # 4
JT = 128          # key tile (partition dim of S^T tiles)
NJT = N // JT     # 32


def build_nc():
    nc = bacc.Bacc()

    xkv_d = nc.dram_tensor("xkv", [C, N], f32, kind="ExternalInput")
    xq_d = nc.dram_tensor("xq", [C, NQ], f32, kind="ExternalInput")
    wqT_d = nc.dram_tensor("wqT", [C, INNER], f32, kind="ExternalInput")
    wkT_d = nc.dram_tensor("wkT", [C, INNER], f32, kind="ExternalInput")
    wvT_d = nc.dram_tensor("wvT", [C, INNER], f32, kind="ExternalInput")
    # per-pair Wout^T with zero rows where `an` has no data:
    # rows 0:32 -> head 2p, rows 64:96 -> head 2p+1
    wo0_d = nc.dram_tensor("wo0", [INNER, C], f32, kind="ExternalInput")
    wo1_d = nc.dram_tensor("wo1", [INNER, C], f32, kind="ExternalInput")
    biasT_d = nc.dram_tensor("biasT", [128, 2], f32, kind="ExternalInput")
    out_d = nc.dram_tensor("out", [C, NQ], f32, kind="ExternalOutput")

    with tile.TileContext(nc) as tc:
        import contextlib

        ctx = contextlib.ExitStack()
        with ctx:
            big = ctx.enter_context(tc.tile_pool(name="big", bufs=1))
            wk = ctx.enter_context(tc.tile_pool(name="wk", bufs=2))
            ptp = ctx.enter_context(tc.tile_pool(name="ptp", bufs=3))
            ps_misc = ctx.enter_context(tc.tile_pool(name="ps_misc", bufs=2, space="PSUM"))
            ps_st = ctx.enter_context(tc.tile_pool(name="ps_st", bufs=2, space="PSUM"))
            ps_acc = ctx.enter_context(tc.tile_pool(name="ps_acc", bufs=2, space="PSUM"))

            # ---- constants / weights ----
            wqT_sb = big.tile([128, 2, INNER], f32)   # [c_part, c_chunk, inner]
            wkT_sb = big.tile([128, 2, INNER], f32)
            wvT_sb = big.tile([128, 2, INNER], f32)
            wo_sb = big.tile([128, 2, C], f32)        # [inner, pair, c]
            bias_sb = big.tile([128, 2], f32)
            ones1_sb = big.tile([128, 128], f32)      # row 0 used as [1,128] ones
            nc.vector.memset(ones1_sb[:], 1.0)
            for cc in range(2):
                nc.sync.dma_start(out=wqT_sb[:, cc, :], in_=wqT_d[128 * cc:128 * (cc + 1), :])
                nc.sync.dma_start(out=wkT_sb[:, cc, :], in_=wkT_d[128 * cc:128 * (cc + 1), :])
                nc.sync.dma_start(out=wvT_sb[:, cc, :], in_=wvT_d[128 * cc:128 * (cc + 1), :])
            nc.sync.dma_start(out=wo_sb[:, 0, :], in_=wo0_d[:])
            nc.sync.dma_start(out=wo_sb[:, 1, :], in_=wo1_d[:])
            nc.sync.dma_start(out=bias_sb[:], in_=biasT_d[:])

            # ---- activations in ----
            xkv_sb = big.tile([128, 2, N], f32)   # [c_part, c_chunk, n]
            xq_sb = big.tile([128, 2, NQ], f32)
            for cc in range(2):
                nc.sync.dma_start(out=xkv_sb[:, cc, :], in_=xkv_d[128 * cc:128 * (cc + 1), :])
                nc.sync.dma_start(out=xq_sb[:, cc, :], in_=xq_d[128 * cc:128 * (cc + 1), :])

            k_sb = big.tile([128, N], f32)     # [inner, n]
            q_sb = big.tile([128, NQ], f32)    # [inner, nq]
            # v^T chunks + ones col: [j0, (jtile, head), 33]; col 32 stays 1.0
            vT3 = big.tile([128, NJT * HEADS, DH + 1], f32)
            nc.vector.memset(vT3[:], 1.0)

            # ---- projections ----
            # k = Wk @ xkv ; q = Wq @ xq   (accumulate over the two C chunks)
            for t in range(N // 512):
                kp = ps_misc.tile([128, 512], f32, tag="misc", name="kp")
                for cc in range(2):
                    nc.tensor.matmul(
                        out=kp[:],
                        lhsT=wkT_sb[:, cc, :],
                        rhs=xkv_sb[:, cc, 512 * t:512 * (t + 1)],
                        start=(cc == 0), stop=(cc == 1),
                    )
                nc.scalar.copy(out=k_sb[:, 512 * t:512 * (t + 1)], in_=kp[:])
            for t in range(NQ // 512):
                qp = ps_misc.tile([128, 512], f32, tag="misc", name="qp")
                for cc in range(2):
                    nc.tensor.matmul(
                        out=qp[:],
                        lhsT=wqT_sb[:, cc, :],
                        rhs=xq_sb[:, cc, 512 * t:512 * (t + 1)],
                        start=(cc == 0), stop=(cc == 1),
                    )
                nc.vector.tensor_copy(out=q_sb[:, 512 * t:512 * (t + 1)], in_=qp[:])
            # vT[n, inner] = x^T @ Wv^T, 128-row tiles of n; 4 tiles per bank,
            # then one strided copy into the 33-col-stride augmented layout
            for T in range(N // 512):
                vp = ps_misc.tile([128, 4, 128], f32, tag="misc", name="vp")
                for t2 in range(4):
                    t = 4 * T + t2
                    for cc in range(2):
                        nc.tensor.matmul(
                            out=vp[:, t2, :],
                            lhsT=xkv_sb[:, cc, 128 * t:128 * (t + 1)],
                            rhs=wvT_sb[:, cc, :],
                            start=(cc == 0), stop=(cc == 1),
                        )
                src = vp.rearrange("p t (h d) -> p (t h) d", d=DH)
                nc.vector.tensor_copy(
                    out=vT3[:, 16 * T:16 * (T + 1), 0:DH], in_=src
                )

            # ---- attention ----
            for qb in range(NQB):
                q0 = QB * qb
                an_list = []
                for p in range(2):
                    # acc bank per (qb, pair): head hh -> A rows 64hh..64hh+32,
                    # denominator row 64hh+32 (ones column of the PV weights)
                    acc = ps_acc.tile([128, QB], f32, tag="acc", name="acc")
                    pv_prev = None
                    for J in range(NJT):
                        st = ps_st.tile([128, 2, QB], f32, tag="st", name="st")
                        for hh in range(2):
                            h = 2 * p + hh
                            nc.tensor.matmul(
                                out=st[:, hh, :],
                                lhsT=k_sb[32 * h:32 * (h + 1), JT * J:JT * (J + 1)],
                                rhs=q_sb[32 * h:32 * (h + 1), q0:q0 + QB],
                                start=True, stop=True,
                                tile_position=(32 * h, 0),
                            )
                        pt = ptp.tile([128, 2, QB], f32, tag="pt", name="pt")
                        nc.scalar.activation(
                            out=pt[:], in_=st[:],
                            func=mybir.ActivationFunctionType.Exp,
                            scale=SCALE,
                        )
                        for hh in range(2):
                            h = 2 * p + hh
                            r0 = 64 * hh
                            mm = nc.tensor.matmul(
                                out=acc[r0:r0 + 33, :],
                                lhsT=vT3[:, HEADS * J + h, :],
                                rhs=pt[:, hh, :],
                                start=(J == 0), stop=(J == NJT - 1),
                                tile_position=(0, r0),
                                skip_group_check=True,
                            )
                            if pv_prev is not None:
                                add_dep_helper(mm.ins, pv_prev.ins, sync=False, reason="pv order")
                            pv_prev = mm
                    # normalize: an rows 64hh..64hh+32 = A_hh / l_hh
                    an = wk.tile([128, QB], f32, tag="an", name="an")
                    nc.vector.memset(an[:], 0.0)
                    for hh in range(2):
                        r0 = 64 * hh
                        lrow = wk.tile([128, QB], f32, tag="lrow", name="lrow")
                        nc.vector.tensor_copy(
                            out=lrow[r0 + 32:r0 + 33, :], in_=acc[r0 + 32:r0 + 33, :]
                        )
                        # K=1 matmul broadcasts the denominator row across all
                        # 128 partitions (lhsT/rhs both live on partition r0+32)
                        lrep = ps_misc.tile([128, QB], f32, tag="misc", name="lrep")
                        nc.tensor.matmul(
                            out=lrep[:],
                            lhsT=ones1_sb[r0 + 32:r0 + 33, :],
                            rhs=lrow[r0 + 32:r0 + 33, :],
                            start=True, stop=True,
                            tile_position=(r0 + 32, 0),
                        )
                        rcp = wk.tile([128, QB], f32, tag="rcp", name="rcp")
                        nc.vector.reciprocal(out=rcp[r0:r0 + 32, :], in_=lrep[r0:r0 + 32, :])
                        nc.vector.tensor_mul(
                            out=an[r0:r0 + 32, :], in0=acc[r0:r0 + 32, :], in1=rcp[r0:r0 + 32, :]
                        )
                    an_list.append(an)
                # out projection + bias
                for cb in range(2):
                    op = ps_misc.tile([128, QB], f32, tag="misc", name="op")
                    for p in range(2):
                        nc.tensor.matmul(
                            out=op[:],
                            lhsT=wo_sb[:, p, 128 * cb:128 * (cb + 1)],
                            rhs=an_list[p][:],
                            start=(p == 0), stop=(p == 1),
                        )
                    ob = wk.tile([128, QB], f32, tag="ob", name="ob")
                    nc.vector.tensor_scalar_add(
                        out=ob[:], in0=op[:], scalar1=bias_sb[:, cb:cb + 1]
                    )
                    nc.sync.dma_start(
                        out=out_d[128 * cb:128 * (cb + 1), q0:q0 + QB], in_=ob[:]
                    )

    nc.compile()
    return nc


_NC_CACHE = []


def _get_nc():
    if not _NC_CACHE:
        _NC_CACHE.append(build_nc())
    return _NC_CACHE[0]


def _make_in_maps(x, Wq, Wk, Wv, Wout, bout):
    xf = np.ascontiguousarray(x.reshape(B, C, N), dtype=np.float32)
    wqT = np.ascontiguousarray(Wq.T, dtype=np.float32)
    wkT = np.ascontiguousarray(Wk.T, dtype=np.float32)
    wvT = np.ascontiguousarray(Wv.T, dtype=np.float32)
    woutT = np.asarray(Wout.T, dtype=np.float32)  # [inner, C]
    wo = []
    for p in range(2):
        m = np.zeros((INNER, C), dtype=np.float32)
        m[0:32] = woutT[64 * p:64 * p + 32]        # head 2p   -> an rows 0:32
        m[64:96] = woutT[64 * p + 32:64 * p + 64]  # head 2p+1 -> an rows 64:96
        wo.append(m)
    biasT = np.ascontiguousarray(bout.reshape(2, 128).T, dtype=np.float32)
    in_maps = []
    for core in range(8):
        b, half = core // 2, core % 2
        q0 = half * NQ
        in_maps.append({
            "xkv": xf[b],
            "xq": np.ascontiguousarray(xf[b][:, q0:q0 + NQ]),
            "wqT": wqT, "wkT": wkT, "wvT": wvT,
            "wo0": wo[0], "wo1": wo[1], "biasT": biasT,
        })
    return in_maps


def kernel(x, Wq, Wk, Wv, Wout, bout):
    nc = _get_nc()
    in_maps = _make_in_maps(x, Wq, Wk, Wv, Wout, bout)
    res = run_bass_kernel_spmd(nc, in_maps, core_ids=list(range(8)))
    out = np.empty((B, C, N), dtype=np.float32)
    for core in range(8):
        b, half = core // 2, core % 2
        q0 = half * NQ
        out[b][:, q0:q0 + NQ] = res.results[core]["out"]
    return out.reshape(B, C, 64, 64)



# revision 6
# speedup vs baseline: 4.2531x; 4.2531x over previous
"""Trainium2 Bass kernel for 4-head spatial self-attention (bf16 pipeline).

Computation (per batch b):
    xf = x[b] reshaped [C=256, n=4096]
    q/k/v = Wq/Wk/Wv @ xf            -> [128, n]   (rows = 4 heads x 32 dims)
    S_h   = (q_h^T k_h) * 32^-0.5    -> [n, n] per head
    P     = exp(S)   (softmax without max-subtraction: logits are O(6))
    A_h   = P_h^T-normalized @ v_h   -> [n, 32]
    out   = Wout @ A + bout          -> [C, n]

Sharding: 8 cores = 4 batches x 2 query-halves. Each core handles all 4 heads
for one batch and 2048 queries vs all 4096 keys; outputs are disjoint slices.

Design notes (cost-model driven):
 - All matmuls run in bf16 (1 PE cycle/output-column vs 4 for fp32).
 - S is computed TRANSPOSED (keys on partitions, queries free), 4 heads packed
   onto PE row strips via tile_position (32h, 0); each head's [128, 512] S^T
   needs its own PSUM bank (probed HW constraint for concurrent row strips),
   so heads go in pairs to 2-bank tiles [128, 2, 512].
 - exp is split across TWO engines: ScalarE computes exact exp -> bf16; DVE
   computes a Schraudolph approximation (round(S*A+B) as int16 IS the bf16
   bit pattern of exp(S*SCALE)); split ratio balances the two engines.
   Softmax renormalization absorbs the ~2% approximation noise.
 - PV runs TRANSPOSED as well: A^T[q,d] = sum_j P^T[j,q]^T v^T[j,d], with the
   512-wide P^T chunk as the STATIONARY operand and the 33-wide v chunk as
   the MOVING operand, accumulating over the 32 key chunks. vT carries an
   extra ones column so A^T column 32 is the softmax denominator -- a
   per-partition scalar, normalized with one reciprocal + broadcast multiply.
   Probed HW constraint: only one OPEN accumulation group per PSUM bank, so
   the 16 groups (4 q-subchunks x 4 heads) run as a sequential tail per
   query block, software-pipelined against the next block's S^T/exp stream
   (the P^T tiles of a block stay resident in SBUF: 64 tiles + slack).
 - an^T -> an via DMA-transpose (16x128 xbar tiles, bf16), then a plain
   [c,q] = Wout^T.T @ an out-projection + bias, DMA'd out per [128, 512].
"""

import numpy as np
import sys

for _p in ("/opt/trn_rl_repo", "/opt/pypackages"):
    if _p not in sys.path:
        sys.path.append(_p)

import ml_dtypes
import concourse.bass as bass
import concourse.tile as tile
from concourse import bacc, mybir
from concourse.tile import add_dep_helper
from concourse.bass_utils import run_bass_kernel_spmd

f32 = mybir.dt.float32
bf16 = mybir.dt.bfloat16
i16 = mybir.dt.int16

B = 4
C = 256
N = 4096          # h*w = 64*64 key positions
NQ = 2048         # queries per core (half batch)
HEADS = 4
DH = 32
INNER = 128
SCALE = DH ** -0.5

QB = 512          # query block (free dim of S^T tiles)
NQB = NQ // QB    # 4
JT = 128          # key tile (partition dim of S^T tiles)
NJT = N // JT     # 32

PT_BUFS = NJT * 2 + 10   # P^T tiles: one block resident + pipeline slack

# Schraudolph bf16 exp: int16(round(S*A_EXP + B_EXP)) bitcast to bf16
A_EXP = SCALE * 128.0 / float(np.log(2.0))
B_EXP = 16256.0 - 5.5

# ScalarE : DVE exp tile split (261:256 scaled) balancing both engines
ACT_FRAC = 143.0 / 256.0


def _use_act(t):
    r = ACT_FRAC
    return int((t + 1) * r) - int(t * r) == 1


def build_nc():
    nc = bacc.Bacc()

    xkv_d = nc.dram_tensor("xkv", [C, N], bf16, kind="ExternalInput")
    wqT_d = nc.dram_tensor("wqT", [C, INNER], bf16, kind="ExternalInput")
    wkT_d = nc.dram_tensor("wkT", [C, INNER], bf16, kind="ExternalInput")
    wvT_d = nc.dram_tensor("wvT", [C, INNER], bf16, kind="ExternalInput")
    woT_d = nc.dram_tensor("woT", [INNER, C], bf16, kind="ExternalInput")
    biasT_d = nc.dram_tensor("biasT", [128, 2], f32, kind="ExternalInput")
    out_d = nc.dram_tensor("out", [C, NQ], f32, kind="ExternalOutput")

    # One program for all 8 cores: the host passes xkv ROLLED so this core's
    # queries sit in columns 0:NQ. Key order is shared by k and v (both come
    # from the same rolled xkv), and softmax sums are order-invariant.
    q0 = 0

    with tile.TileContext(nc) as tc:
        import contextlib

        ctx = contextlib.ExitStack()
        with ctx:
            big = ctx.enter_context(tc.tile_pool(name="big", bufs=1))
            wk = ctx.enter_context(tc.tile_pool(name="wk", bufs=2))
            ptp = ctx.enter_context(tc.tile_pool(name="ptp", bufs=PT_BUFS))
            ps_st = ctx.enter_context(tc.tile_pool(name="ps_st", bufs=3, space="PSUM"))
            ps_acc = ctx.enter_context(tc.tile_pool(name="ps_acc", bufs=2, space="PSUM"))

            # ---- constants / weights ----
            wqT_sb = big.tile([128, 2, INNER], bf16)   # [c_part, c_chunk, inner]
            wkT_sb = big.tile([128, 2, INNER], bf16)
            wvT_sb = big.tile([128, 2, INNER], bf16)
            woT_sb = big.tile([128, C], bf16)          # [inner, c]
            bias_sb = big.tile([128, 2], f32)
            for cc in range(2):
                nc.sync.dma_start(out=wqT_sb[:, cc, :], in_=wqT_d[128 * cc:128 * (cc + 1), :])
                nc.sync.dma_start(out=wkT_sb[:, cc, :], in_=wkT_d[128 * cc:128 * (cc + 1), :])
                nc.sync.dma_start(out=wvT_sb[:, cc, :], in_=wvT_d[128 * cc:128 * (cc + 1), :])
            nc.sync.dma_start(out=woT_sb[:], in_=woT_d[:])
            nc.sync.dma_start(out=bias_sb[:], in_=biasT_d[:])

            # ---- activations in ----
            xkv_sb = big.tile([128, 2, N], bf16)   # [c_part, c_chunk, n]
            for cc in range(2):
                nc.sync.dma_start(out=xkv_sb[:, cc, :], in_=xkv_d[128 * cc:128 * (cc + 1), :])

            k_sb = big.tile([128, N], bf16)     # [inner, n]
            q_sb = big.tile([128, NQ], bf16)    # [inner, nq]
            # v^T chunks + ones col: [j0, (jtile, head), 33]; col 32 = 1.0
            vT3 = big.tile([128, NJT * HEADS, DH + 1], bf16)
            nc.gpsimd.memset(vT3[:, :, 32:33], 1.0)

            # ---- projections ----
            # interleave k (ScalarE copies) with q/v (DVE copies)
            for t in range(N // 512):
                kp = ps_st.tile([128, 512], f32, tag="st", name="kp")
                for cc in range(2):
                    nc.tensor.matmul(
                        out=kp[:],
                        lhsT=wkT_sb[:, cc, :],
                        rhs=xkv_sb[:, cc, 512 * t:512 * (t + 1)],
                        start=(cc == 0), stop=(cc == 1),
                    )
                nc.scalar.copy(out=k_sb[:, 512 * t:512 * (t + 1)], in_=kp[:])

                if t < NQ // 512:
                    qp = ps_st.tile([128, 512], f32, tag="st", name="qp")
                    for cc in range(2):
                        nc.tensor.matmul(
                            out=qp[:],
                            lhsT=wqT_sb[:, cc, :],
                            rhs=xkv_sb[:, cc, q0 + 512 * t:q0 + 512 * (t + 1)],
                            start=(cc == 0), stop=(cc == 1),
                        )
                    nc.vector.tensor_copy(out=q_sb[:, 512 * t:512 * (t + 1)], in_=qp[:])

                # vT[n, inner] = x^T @ Wv^T, 128-row tiles of n
                vp = ps_st.tile([128, 4, 128], f32, tag="st", name="vp")
                for t2 in range(4):
                    j = 4 * t + t2
                    for cc in range(2):
                        nc.tensor.matmul(
                            out=vp[:, t2, :],
                            lhsT=xkv_sb[:, cc, 128 * j:128 * (j + 1)],
                            rhs=wvT_sb[:, cc, :],
                            start=(cc == 0), stop=(cc == 1),
                        )
                src = vp.rearrange("p t (h d) -> p (t h) d", d=DH)
                nc.vector.tensor_copy(
                    out=vT3[:, 16 * t:16 * (t + 1), 0:DH], in_=src
                )

            # ---- attention ----
            pt_tiles = {}     # (qb, J, p) -> pt AP
            exp_idx = [0]

            def emit_j(qb, J):
                for p in range(2):
                    st = ps_st.tile([128, 2, QB], f32, tag="st", name="st")
                    for hh in range(2):
                        h = 2 * p + hh
                        nc.tensor.matmul(
                            out=st[:, hh, :],
                            lhsT=k_sb[32 * h:32 * (h + 1), JT * J:JT * (J + 1)],
                            rhs=q_sb[32 * h:32 * (h + 1), QB * qb:QB * (qb + 1)],
                            start=True, stop=True,
                            tile_position=(32 * h, 0),
                        )
                    pt = ptp.tile([128, 2, QB], bf16, tag="pt", name="pt")
                    pt_tiles[(qb, J, p)] = pt
                    t = exp_idx[0]
                    exp_idx[0] += 1
                    if _use_act(t):
                        nc.scalar.activation(
                            out=pt[:], in_=st[:],
                            func=mybir.ActivationFunctionType.Exp,
                            scale=SCALE,
                        )
                    else:
                        nc.vector.tensor_scalar(
                            out=pt.bitcast(i16)[:], in0=st[:],
                            scalar1=A_EXP, scalar2=B_EXP,
                            op0=mybir.AluOpType.mult, op1=mybir.AluOpType.add,
                        )

            def emit_tail(qb):
                # PV^T: 16 sequential accumulation groups (one open per bank)
                # layout [q, i, h, d]: per q-subchunk pair i, head-major cols
                acc01 = ps_acc.tile([128, 2, HEADS, DH + 1], f32, tag="acc", name="acc01")
                acc23 = ps_acc.tile([128, 2, HEADS, DH + 1], f32, tag="acc", name="acc23")
                prev = None
                for i in range(4):
                    accT = acc01 if i < 2 else acc23
                    ii = i % 2
                    for h in range(4):
                        p, hh = h // 2, h % 2
                        out_ap = accT[:, ii, h, :]
                        for J in range(NJT):
                            mm = nc.tensor.matmul(
                                out=out_ap,
                                lhsT=pt_tiles[(qb, J, p)][:, hh, 128 * i:128 * (i + 1)],
                                rhs=vT3[:, HEADS * J + h, :],
                                start=(J == 0), stop=(J == NJT - 1),
                                skip_group_check=True,
                            )
                            if prev is not None:
                                add_dep_helper(mm.ins, prev.ins, sync=False, reason="pv order")
                            prev = mm
                # normalize: an^T[q, (h d)] = A^T[q, h, d] / A^T[q, h, 32]
                anTs = []
                for accT in (acc01, acc23):
                    rcp = wk.tile([128, 2, 4], f32, tag="rcp", name="rcp")
                    nc.vector.reciprocal(out=rcp[:], in_=accT[:, :, :, DH])
                    anT = wk.tile([128, 2, 4, DH], bf16, tag="anT", name="anT")
                    nc.vector.tensor_mul(
                        out=anT[:],
                        in0=accT[:, :, :, 0:DH],
                        in1=rcp.unsqueeze(3).broadcast_to((128, 2, 4, DH)),
                    )
                    anTs.append(anT)
                # DMA transpose an^T -> an [inner, q]
                an = wk.tile([128, QB], bf16, tag="an", name="an")
                for i in range(4):
                    anT = anTs[i // 2]
                    ii = i % 2
                    nc.sync.dma_start_transpose(
                        out=an[:, 128 * i:128 * (i + 1)],
                        in_=anT[:, ii].rearrange("q h d -> q (h d)"),
                    )
                # out projection + bias
                for cb in range(2):
                    op = ps_acc.tile([128, QB], f32, tag="acc", name="op")
                    nc.tensor.matmul(
                        out=op[:],
                        lhsT=woT_sb[:, 128 * cb:128 * (cb + 1)],
                        rhs=an[:],
                        start=True, stop=True,
                    )
                    ob = wk.tile([128, QB], f32, tag="ob", name="ob")
                    nc.vector.tensor_scalar_add(
                        out=ob[:], in0=op[:], scalar1=bias_sb[:, cb:cb + 1]
                    )
                    nc.sync.dma_start(
                        out=out_d[128 * cb:128 * (cb + 1), QB * qb:QB * (qb + 1)],
                        in_=ob[:],
                    )

            for qb in range(NQB):
                for J in range(NJT):
                    emit_j(qb, J)
                    if J == 2 and qb > 0:
                        emit_tail(qb - 1)
                if qb == NQB - 1:
                    emit_tail(qb)

    nc.compile()
    return nc


_NC_CACHE = []


def _get_nc():
    if not _NC_CACHE:
        _NC_CACHE.append(build_nc())
    return _NC_CACHE[0]


def _make_in_maps(x, Wq, Wk, Wv, Wout, bout):
    bfl = ml_dtypes.bfloat16
    xf = np.asarray(x, dtype=np.float32).reshape(B, C, N)
    wqT = np.ascontiguousarray(np.asarray(Wq, np.float32).T).astype(bfl)
    wkT = np.ascontiguousarray(np.asarray(Wk, np.float32).T).astype(bfl)
    wvT = np.ascontiguousarray(np.asarray(Wv, np.float32).T).astype(bfl)
    woT = np.ascontiguousarray(np.asarray(Wout, np.float32).T).astype(bfl)
    biasT = np.ascontiguousarray(
        np.asarray(bout, np.float32).reshape(2, 128).T
    ).astype(np.float32)
    in_maps = []
    for core in range(8):
        b, half = core // 2, core % 2
        q0 = half * NQ
        # roll keys so this core's queries occupy columns 0:NQ; key order is
        # shared by k and v so softmax/PV are unaffected.
        xroll = np.roll(xf[b], -q0, axis=1) if q0 else xf[b]
        in_maps.append({
            "xkv": np.ascontiguousarray(xroll).astype(bfl),
            "wqT": wqT, "wkT": wkT, "wvT": wvT,
            "woT": woT, "biasT": biasT,
        })
    return in_maps


def kernel(x, Wq, Wk, Wv, Wout, bout):
    nc = _get_nc()
    in_maps = _make_in_maps(x, Wq, Wk, Wv, Wout, bout)
    res = run_bass_kernel_spmd(nc, in_maps, core_ids=list(range(8)))
    out = np.empty((B, C, N), dtype=np.float32)
    for core in range(8):
        b, half = core // 2, core % 2
        q0 = half * NQ
        out[b][:, q0:q0 + NQ] = res.results[core]["out"]
    return out.reshape(B, C, 64, 64)
